# revision 1
# baseline (speedup 1.0000x reference)
"""Trainium2 Bass kernel for nn_DifferentiableRobotModel (self-collision link
distances from batched forward kinematics).

Pure data parallel over the batch (rollout) dim: 8192 rollouts -> 1024/core
on 8 NeuronCores. All FK params / sphere tables / masks are tiny and
replicated.

Per-core pipeline (B_c = 1024 batches, 128 spheres = 16 links x 8):
  1. FK on DVE, batch-on-partitions: serial chain over 16 links using
     M_l(q) = P_l + sin(q) Q_l + (1-cos q) S_l (P,Q,S host-precomputed),
     producing per link [R|t], u = R^T t, tau = ||t||^2 in a "S" layout
     [128 batches, 17 slots x 4 rows x 12 planes].
  2. PE transposes S -> M_ALL [66, 10 * B_c]: per batch a [66,10] matrix
     ("M_cat2T") whose 10 columns project onto sphere attributes.
  3. Per batch: CT = M_cat2T^T @ BD2 -> [10, 128] in PSUM, where BD2 [66,128]
     is the constant block-diagonal sphere table. Rows: [cx,cy,cz,sq,1] (T1)
     and [-2cx,-2cy,-2cz,1,sq] (T2), sq = ||c_j||^2 + EPS/1.
  4. Gram: d2 = T1^T @ T2 = sq_i + sq_j - 2 c_i.c_j + EPS  [128,128] PSUM.
  5. ACT: s = sqrt(d2) (EPS keeps the argument positive, incl. the diagonal).
  6. DVE tensor_tensor_reduce per batch: Z[i] = max_j (RJM[i,j] - s[i,j]),
     RJM[i,j] = r_j where link pair allowed else -1e9 (mask folded in).
  7. Z + r_i, PE transpose -> [batch, sphere], grouped max over each link's
     8 spheres -> out [batch, 16].
"""
import sys
import numpy as np

sys.path.insert(0, "/opt/trn_rl_repo")

import concourse.bass as bass  # noqa: E402
import concourse.tile as tile  # noqa: E402
from concourse import bacc, mybir  # noqa: E402
from contextlib import ExitStack  # noqa: E402

F32 = mybir.dt.float32
AF = mybir.ActivationFunctionType
ALU = mybir.AluOpType
AX = mybir.AxisListType

B, L, NS = 8192, 16, 8
N = L * NS              # 128 spheres
NCORES = 8
EPS = np.float32(4e-6)  # d2 positivity shift (masked-pair sqrt stays finite)
MASKVAL = np.float32(-1e9)
GRP = 8                 # batches per main-loop group (PSUM bank budget)

_CACHE = {}


# ---------------------------------------------------------------- host consts
def _host_consts(fixed_rot, fixed_trans, joint_axes, link_spheres,
                 collision_mask, bc):
    f32 = np.float32
    ax = np.asarray(joint_axes, f32)
    K = np.zeros((L, 3, 3), f32)
    K[:, 0, 1], K[:, 0, 2] = -ax[:, 2], ax[:, 1]
    K[:, 1, 0], K[:, 1, 2] = ax[:, 2], -ax[:, 0]
    K[:, 2, 0], K[:, 2, 1] = -ax[:, 1], ax[:, 0]
    K2 = np.einsum("lij,ljk->lik", K, K).astype(f32)
    A = np.asarray(fixed_rot, f32)
    P = A
    Q = np.einsum("lij,ljk->lik", A, K).astype(f32)
    S = np.einsum("lij,ljk->lik", A, K2).astype(f32)

    # pqs [128, 432]: sections P/Q/S, col sec*144 + l*9 + 3j+k, replicated rows
    pqs = np.zeros((128, 432), f32)
    for sec, Mx in enumerate((P, Q, S)):
        pqs[:, sec * 144:(sec + 1) * 144] = Mx.reshape(1, L * 9)
    fb = np.zeros((128, 48), f32)
    fb[:, :] = np.asarray(fixed_trans, f32).reshape(1, L * 3)

    x = np.asarray(link_spheres, f32)[..., :3]           # [L,NS,3]
    r = np.asarray(link_spheres, f32)[..., 3].reshape(N)
    xsq = (x ** 2).sum(-1).reshape(N)

    bd2 = np.zeros((66, N), f32)
    for l in range(L):
        for k in range(3):
            bd2[4 * l + k, l * NS:(l + 1) * NS] = x[l, :, k]
        bd2[4 * l + 3, l * NS:(l + 1) * NS] = 1.0
    bd2[64] = xsq
    bd2[65] = 1.0

    li = np.arange(N) // NS
    allowed = np.abs(li[:, None] - li[None, :]) > 1
    cm = np.asarray(collision_mask)[li[:, None], li[None, :]]
    allowed = allowed & cm
    rjm = np.where(allowed, r[None, :], MASKVAL).astype(f32)
    rcol = r.reshape(N, 1).astype(f32)

    # mrows [2, 10*bc]: M_cat2T const rows 64/65, plane-major (plane m at m*bc)
    mrows = np.zeros((2, 10 * bc), f32)
    mrows[0, 3 * bc:4 * bc] = 1.0          # row64 plane3 (sq <- ||x||^2)
    mrows[0, 9 * bc:10 * bc] = 1.0         # row64 plane9
    mrows[1, 4 * bc:5 * bc] = 1.0          # row65 plane4 (T1 ones)
    mrows[1, 8 * bc:9 * bc] = 1.0          # row65 plane8 (T2 ones)
    mrows[1, 3 * bc:4 * bc] = EPS          # d2 + EPS
    mrows[1, 9 * bc:10 * bc] = EPS
    ident = np.eye(128, dtype=f32)
    pio2 = np.full((128, 1), np.pi / 2, f32)
    return dict(pqs=pqs, fb=fb, bd2=bd2, rjm=rjm, rcol=rcol, mrows=mrows,
                ident=ident, pio2=pio2)


# ---------------------------------------------------------------- device build
def _build_nc(nt):
    """Build + compile the per-core Bass module for nt tiles of 128 batches."""
    bc = nt * 128
    nc = bacc.Bacc("TRN2", target_bir_lowering=False, debug=False,
                   num_devices=NCORES)

    q_d = nc.dram_tensor("q", [bc, L], F32, kind="ExternalInput").ap()
    pqs_d = nc.dram_tensor("pqs", [128, 432], F32, kind="ExternalInput").ap()
    fb_d = nc.dram_tensor("fb", [128, 48], F32, kind="ExternalInput").ap()
    bd2_d = nc.dram_tensor("bd2", [66, N], F32, kind="ExternalInput").ap()
    rjm_d = nc.dram_tensor("rjm", [N, N], F32, kind="ExternalInput").ap()
    rcol_d = nc.dram_tensor("rcol", [N, 1], F32, kind="ExternalInput").ap()
    mrows_d = nc.dram_tensor("mrows", [2, 10 * bc], F32,
                             kind="ExternalInput").ap()
    ident_d = nc.dram_tensor("ident", [128, 128], F32,
                             kind="ExternalInput").ap()
    pio2_d = nc.dram_tensor("pio2", [128, 1], F32, kind="ExternalInput").ap()
    out_d = nc.dram_tensor("out", [bc, L], F32, kind="ExternalOutput").ap()

    # persistent SBUF tensors (custom APs need integer offsets)
    qsb = nc.alloc_sbuf_tensor("qsb", [128, 16 * nt], F32).ap()
    sinb = nc.alloc_sbuf_tensor("sinb", [128, 16 * nt], F32).ap()
    cosb = nc.alloc_sbuf_tensor("cosb", [128, 16 * nt], F32).ap()
    omcb = nc.alloc_sbuf_tensor("omcb", [128, 16 * nt], F32).ap()
    pqs = nc.alloc_sbuf_tensor("pqs_sb", [128, 432], F32).ap()
    fbt = nc.alloc_sbuf_tensor("fb_sb", [128, 48], F32).ap()
    bd2 = nc.alloc_sbuf_tensor("bd2_sb", [66, N], F32).ap()
    rjm = nc.alloc_sbuf_tensor("rjm_sb", [N, N], F32).ap()
    rcol = nc.alloc_sbuf_tensor("rcol_sb", [N, 1], F32).ap()
    ident = nc.alloc_sbuf_tensor("ident_sb", [128, 128], F32).ap()
    pio2 = nc.alloc_sbuf_tensor("pio2_sb", [128, 1], F32).ap()
    # FK state: col = t*816 + (slot*4 + k)*12 + m ; slot 0 = identity pose
    SP = 816
    sfk = nc.alloc_sbuf_tensor("sfk", [128, SP * nt], F32).ap()
    mw = nc.alloc_sbuf_tensor("mw", [128, 144 * nt], F32).ap()
    mw2 = nc.alloc_sbuf_tensor("mw2", [128, 144 * nt], F32).ap()
    tscr = nc.alloc_sbuf_tensor("tscr", [128, 9 * nt], F32).ap()
    tsc2 = nc.alloc_sbuf_tensor("tsc2", [128, 3 * nt], F32).ap()
    # M_ALL [66, 10*bc], plane-major: col = m*bc + b
    mall = nc.alloc_sbuf_tensor("mall", [66, 10 * bc], F32).ap()

    def cap(base, offset, dims):
        """Custom AP on a persistent tensor: dims = [[step,count],...] (free)."""
        pitch = base.tensor.shape[-1]
        nparts = base.tensor.shape[0]
        return bass.AP(tensor=base.tensor, offset=offset,
                       ap=[[pitch, nparts]] + list(dims))

    def capp(base, prow, nrow, offset, dims):
        """Custom AP with partition sub-range [prow, prow+nrow)."""
        pitch = base.tensor.shape[-1]
        return bass.AP(tensor=base.tensor, offset=prow * pitch + offset,
                       ap=[[pitch, nrow]] + list(dims))

    with tile.TileContext(nc) as tc, ExitStack() as ctx:
        ctpool = ctx.enter_context(tc.tile_pool(name="ct", bufs=1,
                                                space="PSUM"))
        grpool = ctx.enter_context(tc.tile_pool(name="gram", bufs=2,
                                                space="PSUM"))
        trmpool = ctx.enter_context(tc.tile_pool(name="trm", bufs=1,
                                                 space="PSUM"))
        trzpool = ctx.enter_context(tc.tile_pool(name="trz", bufs=1,
                                                 space="PSUM"))
        ctsbp = ctx.enter_context(tc.tile_pool(name="ctsb", bufs=2))
        t2sbp = ctx.enter_context(tc.tile_pool(name="t2sb", bufs=2))
        spool = ctx.enter_context(tc.tile_pool(name="spool", bufs=3))
        scrp = ctx.enter_context(tc.tile_pool(name="scr", bufs=2))
        zpool = ctx.enter_context(tc.tile_pool(name="z", bufs=2))
        z2pool = ctx.enter_context(tc.tile_pool(name="z2", bufs=2))
        outp = ctx.enter_context(tc.tile_pool(name="outsb", bufs=2))

        # ---- input DMAs
        nc.sync.dma_start(pqs, pqs_d)
        nc.sync.dma_start(fbt, fb_d)
        nc.sync.dma_start(bd2, bd2_d)
        nc.sync.dma_start(rjm, rjm_d)
        nc.sync.dma_start(rcol, rcol_d)
        nc.sync.dma_start(ident, ident_d)
        nc.sync.dma_start(pio2, pio2_d)
        nc.sync.dma_start(capp(mall, 64, 2, 0, [[1, 10 * bc]]), mrows_d)
        for t in range(nt):
            nc.sync.dma_start(cap(qsb, 16 * t, [[1, 16]]),
                              q_d[128 * t:128 * (t + 1), :])

        # ---- sin / cos / (1-cos)
        nc.scalar.activation(sinb, qsb, AF.Sin)
        # 1 - cos(q) = 2 sin^2(q/2); Sin LUT domain is [-pi, pi]
        nc.scalar.activation(cosb, qsb, AF.Sin, scale=0.5)
        nc.vector.tensor_mul(omcb, cosb, cosb)
        nc.vector.tensor_scalar_mul(omcb, omcb, 2.0)

        # ---- zero-fill: slot0 of sfk, plus m4/m8 cols of all link slots
        nc.vector.memset(cap(sfk, 0, [[SP, nt], [1, 48]]), 0.0)
        nc.vector.memset(cap(sfk, 0, [[SP, nt], [13, 3]]), 1.0)  # I diag
        nc.vector.memset(cap(sfk, 48 + 4, [[SP, nt], [48, 16], [12, 4]]), 0.0)
        nc.vector.memset(cap(sfk, 48 + 8, [[SP, nt], [48, 16], [12, 4]]), 0.0)
        # zero planes m4/m8 of mall rows 0..63
        nc.scalar.memzero(capp(mall, 0, 64, 4 * bc, [[1, bc]]))
        nc.scalar.memzero(capp(mall, 0, 64, 8 * bc, [[1, bc]]))

        # ---- M_l = P + sin*Q + (1-cos)*S for all links: mw[(t,l,(j,k))]
        mdims = [[144, nt], [9, L], [1, 9]]
        sdims = [[16, nt], [1, L], [0, 9]]
        nc.vector.tensor_mul(cap(mw, 0, mdims), cap(pqs, 144, [[0, nt]] + mdims[1:]),
                             cap(sinb, 0, sdims))
        nc.vector.tensor_mul(cap(mw2, 0, mdims), cap(pqs, 288, [[0, nt]] + mdims[1:]),
                             cap(omcb, 0, sdims))
        nc.vector.tensor_add(mw, mw, mw2)
        nc.vector.tensor_add(cap(mw, 0, mdims), cap(mw, 0, mdims),
                             cap(pqs, 0, [[0, nt]] + mdims[1:]))

        # ---- FK serial chain
        for l in range(L):
            sp, s_ = 48 * l, 48 * (l + 1)     # prev slot, this slot
            # R_l[i,k] = sum_j Rp[i,j] * M[j,k]   (dims iterate t,k,i)
            outR = cap(sfk, s_, [[SP, nt], [12, 3], [1, 3]])
            tmpR = cap(tscr, 0, [[9, nt], [3, 3], [1, 3]])
            for j in range(3):
                i0 = cap(sfk, sp + 12 * j, [[SP, nt], [0, 3], [1, 3]])
                i1 = cap(mw, 9 * l + 3 * j, [[144, nt], [1, 3], [0, 3]])
                if j == 0:
                    nc.vector.tensor_mul(outR, i0, i1)
                else:
                    nc.vector.tensor_mul(tmpR, i0, i1)
                    nc.vector.tensor_add(outR, outR, tmpR)
            # t_l = t_p + Rp @ ftrans_l   (tscr[(t,i,j)] = Rp[i,j]*f[j])
            nc.vector.tensor_mul(cap(tscr, 0, [[9, nt], [3, 3], [1, 3]]),
                                 cap(sfk, sp, [[SP, nt], [1, 3], [12, 3]]),
                                 cap(fbt, 3 * l, [[0, nt], [0, 3], [1, 3]]))
            nc.vector.reduce_sum(cap(tsc2, 0, [[3, nt], [1, 3]]),
                                 cap(tscr, 0, [[9, nt], [3, 3], [1, 3]]),
                                 axis=AX.X)
            nc.vector.tensor_add(cap(sfk, s_ + 36, [[SP, nt], [1, 3]]),
                                 cap(sfk, sp + 36, [[SP, nt], [1, 3]]),
                                 cap(tsc2, 0, [[3, nt], [1, 3]]))
            # u_k = sum_i R[i,k] t[i]  -> col m3 of rows (l,k)
            nc.vector.tensor_mul(cap(tscr, 0, [[9, nt], [3, 3], [1, 3]]),
                                 cap(sfk, s_, [[SP, nt], [12, 3], [1, 3]]),
                                 cap(sfk, s_ + 36, [[SP, nt], [0, 3], [1, 3]]))
            nc.vector.reduce_sum(cap(sfk, s_ + 3, [[SP, nt], [12, 3]]),
                                 cap(tscr, 0, [[9, nt], [3, 3], [1, 3]]),
                                 axis=AX.X)
            # tau = sum_i t[i]^2 -> col m3 of row (l,3)
            nc.vector.tensor_mul(cap(tscr, 0, [[3, nt], [1, 3]]),
                                 cap(sfk, s_ + 36, [[SP, nt], [1, 3]]),
                                 cap(sfk, s_ + 36, [[SP, nt], [1, 3]]))
            nc.vector.reduce_sum(cap(sfk, s_ + 39, [[SP, nt], [1, 1]]),
                                 cap(tscr, 0, [[3, nt], [1, 3]]),
                                 axis=AX.X)

        # ---- post: double u, -2 planes, sq dup (slots 1..16)
        nc.vector.tensor_scalar_mul(
            cap(sfk, 48 + 3, [[SP, nt], [48, L], [12, 3]]),
            cap(sfk, 48 + 3, [[SP, nt], [48, L], [12, 3]]), 2.0)
        nc.vector.tensor_scalar_mul(
            cap(sfk, 48 + 5, [[SP, nt], [48, L], [12, 4], [1, 3]]),
            cap(sfk, 48 + 0, [[SP, nt], [48, L], [12, 4], [1, 3]]), -2.0)
        nc.vector.tensor_copy(
            cap(sfk, 48 + 9, [[SP, nt], [48, L], [12, 4]]),
            cap(sfk, 48 + 3, [[SP, nt], [48, L], [12, 4]]))

        # ---- PE transposes: S[(slot,k) rows, m] -> M_ALL[66, m*bc + b]
        for t in range(nt):
            for planes in ((0, 1, 2, 3), (5, 6, 7), (9,)):
                npl = len(planes)
                trm = trmpool.tile([64, 128 * npl], F32)
                for qi, m in enumerate(planes):
                    nc.tensor.transpose(
                        trm[:, 128 * qi:128 * (qi + 1)],
                        cap(sfk, SP * t + 48 + m, [[48, 16], [12, 4]]),
                        ident)
                nc.scalar.copy(
                    capp(mall, 0, 64, planes[0] * bc + 128 * t,
                         [[bc, npl], [1, 128]]),
                    trm[:, :])

        # ---- main loop: groups of GRP batches
        ngrp = bc // GRP
        z = None
        for g in range(ngrp):
            t = (g * GRP) // 128
            if g % (128 // GRP) == 0:
                z = zpool.tile([128, 128], F32)
            ct = ctpool.tile([10, GRP * 128], F32)
            for bb in range(GRP):
                b = g * GRP + bb
                nc.tensor.matmul(
                    ct[0:10, 128 * bb:128 * (bb + 1)],
                    capp(mall, 0, 66, b, [[bc, 10]]),
                    bd2[0:66, :])
            ctsb = ctsbp.tile([10, GRP * 128], F32)
            nc.scalar.copy(ctsb[:, :], ct[0:10, :])
            t2sb = t2sbp.tile([5, GRP * 128], F32)
            nc.sync.dma_start(t2sb[0:5, :], ctsb[5:10, :])
            gram = grpool.tile([128, GRP * 128], F32)
            for bb in range(GRP):
                nc.tensor.matmul(
                    gram[:, 128 * bb:128 * (bb + 1)],
                    ctsb[0:5, 128 * bb:128 * (bb + 1)],
                    t2sb[0:5, 128 * bb:128 * (bb + 1)])
            s_t = spool.tile([128, GRP * 128], F32)
            nc.scalar.activation(s_t[:, :], gram[:, :], AF.Sqrt)
            # y = RJM - s (mask+radius folded into RJM), then max over j
            y_t = scrp.tile([128, GRP * 128], F32)
            nc.vector.tensor_sub(
                y_t[:, :],
                cap(rjm, 0, [[0, GRP], [1, 128]]),
                s_t[:, :])
            gb0 = (g * GRP) % 128
            nc.vector.tensor_reduce(
                z[:, gb0:gb0 + GRP],
                y_t[:, :].rearrange("p (g j) -> p g j", j=128),
                axis=AX.X, op=ALU.max)
            if g % (128 // GRP) == (128 // GRP) - 1:
                z2 = z2pool.tile([128, 128], F32)
                nc.vector.tensor_scalar_add(z2[:, :], z[:, :], rcol[0:128, 0:1])
                trz = trzpool.tile([128, 128], F32)
                nc.tensor.transpose(trz[:, :], z2[:, :], ident)
                osb = outp.tile([128, L], F32)
                nc.vector.tensor_reduce(
                    osb[:, :], trz[:, :].rearrange("p (a b) -> p a b", a=L),
                    axis=AX.X, op=ALU.max)
                nc.sync.dma_start(out_d[128 * t:128 * (t + 1), :], osb[:, :])

    nc.compile()
    return nc


def get_nc(nt):
    key = ("nc", nt)
    if key not in _CACHE:
        _CACHE[key] = _build_nc(nt)
    return _CACHE[key]


# ---------------------------------------------------------------- entry point
def kernel(q, fixed_rot, fixed_trans, joint_axes, link_spheres,
           collision_mask):
    from concourse.bass_utils import run_bass_kernel_spmd

    q = np.asarray(q, np.float32)
    bc = B // NCORES
    nt = bc // 128
    consts = _host_consts(fixed_rot, fixed_trans, joint_axes, link_spheres,
                          collision_mask, bc)
    nc = get_nc(nt)
    in_maps = []
    for c in range(NCORES):
        m = {"q": np.ascontiguousarray(q[c * bc:(c + 1) * bc]),
             "pqs": consts["pqs"], "fb": consts["fb"], "bd2": consts["bd2"],
             "rjm": consts["rjm"], "rcol": consts["rcol"],
             "mrows": consts["mrows"], "ident": consts["ident"],
             "pio2": consts["pio2"]}
        in_maps.append(m)
    res = run_bass_kernel_spmd(nc, in_maps, list(range(NCORES)))
    out = np.concatenate([res.results[c]["out"] for c in range(NCORES)],
                         axis=0)
    return out.astype(np.float32)

